# revision 26
# baseline (speedup 1.0000x reference)
"""Trainium2 Bass kernel for the pre-norm causal attention sublayer.

Reference computation (fp32):
    y = layernorm(x, ln_w, ln_b)                      [b, s, d]
    q,k,v = per-head projections of y                 [b, h, s, e]
    attn = causal_softmax(q k^T / sqrt(e)) @ v        [b, s, h*e]
    out = attn @ wo + x

Sharding over 8 cores: batch (2-way) x heads (4-way tensor parallel).
Core c handles batch c//4 and heads 4*(c%4) .. 4*(c%4)+3.

Per-core pipeline, interleaved per s-group g (4 s-tiles = 512 rows):
  A(g) LN stats in natural [s, d] layout (DVE free-axis reduces, stats from
       raw sums: var = E[x^2]-E[x]^2), normalize via one tensor_scalar,
       PE-transpose 128x128 tiles -> yT_g [d, 512] (per-group, recycled).
       ln_w/ln_b are folded into the projection weights host-side.
  B(g) qT,kT [he, s-cols of g] via matmul(lhsT=w chunk, rhs=yT_g chunk) +
       per-partition bias; v natural [t, he] via matmul(lhsT=yT_g chunk,
       rhs=wv chunk) + ones-outer-product bias; v stored with a ones column
       per head ([t, 4*65]) so the attention matmul also emits the softmax
       denominator.
  C(j=g) per head: scores^T tiles [t=128, s=512] (K=64), exp on ScalarE
       (scale 1/8; no max-subtraction needed at these magnitudes), causal
       masking of diagonal tiles via affine_select on GpSimd, attnU^T[65,512]
       accumulation (K=128).  Normalize with reciprocal of row 64 broadcast
       across partitions by a K=1 PE outer product.
  D(j) AllGather (groups [[0..3],[4..7]]) of attn^T -> full [1024, 512].
  E(j) out[s-tile, cols] = attn^T.T @ wo[:, col shard] + x residual; each
       core owns 256 output columns; host concatenates.

All matmuls run on the float32r PE path (fp32 storage, ~1 cycle/row).
"""

import numpy as np
from contextlib import ExitStack

import concourse.bass as bass
import concourse.bacc as bacc
import concourse.mybir as mybir
import concourse.tile as tile
from concourse.bass_utils import run_bass_kernel_spmd

F32 = mybir.dt.float32
F32R = mybir.dt.float32r
AF = mybir.ActivationFunctionType
ALU = mybir.AluOpType

B, S, D, H, E = 2, 2048, 1024, 16, 64
HPC = 4                      # heads per core
COLS = 256                   # output columns per core
EPS = 1e-5
PT = 128                     # partition tile
SC = 512                     # s-chunk
NST = S // PT                # 16
NSC = S // SC                # 4
NDC = D // PT                # 8
GROUPS = [[0, 1, 2, 3], [4, 5, 6, 7]]


def build_program(collective=True):
    nd = 8 if collective else 1
    nc = bacc.Bacc("TRN2", target_bir_lowering=False, debug=False, num_devices=nd)

    x = nc.dram_tensor("x", [S, D], F32, kind="ExternalInput")
    # weights arrive pre-chunked from host: [128, 8*256], d-chunk c at cols 256c
    wq = nc.dram_tensor("wq", [PT, NDC * 256], F32R, kind="ExternalInput")
    wk = nc.dram_tensor("wk", [PT, NDC * 256], F32R, kind="ExternalInput")
    wv = nc.dram_tensor("wv", [PT, NDC * 256], F32R, kind="ExternalInput")
    wo = nc.dram_tensor("wo", [PT, NDC * 256], F32R, kind="ExternalInput")
    cq = nc.dram_tensor("cq", [PT, 2], F32, kind="ExternalInput")
    ck = nc.dram_tensor("ck", [PT, 2], F32, kind="ExternalInput")
    cv = nc.dram_tensor("cv", [1, HPC * E], F32R, kind="ExternalInput")
    xres = nc.dram_tensor("xres", [S, COLS], F32, kind="ExternalInput")
    ones_in = nc.dram_tensor("ones_in", [1, PT], F32R, kind="ExternalInput")
    vinit = nc.dram_tensor("vinit", [PT, HPC * (E + 1)], F32R, kind="ExternalInput")
    ident = nc.dram_tensor("ident", [PT, PT], F32, kind="ExternalInput")

    out = nc.dram_tensor("out", [S, COLS], F32, kind="ExternalOutput")

    with tile.TileContext(nc) as tc, ExitStack() as top:
        pc = top.enter_context(tc.tile_pool(name="persist", bufs=1))
        pD = top.enter_context(tc.tile_pool(name="cc", bufs=1, space="DRAM"))
        cc_in = [
            pD.tile([HPC * E, SC], F32R, tag=f"cci{j}", name=f"cc_in_{j}")
            for j in range(NSC)
        ]
        cc_out = [
            pD.tile([D, SC], F32R, tag=f"cco{j}", name=f"cc_out_{j}")
            for j in range(NSC)
        ]

        ones_sb = pc.tile([1, PT], F32R, tag="ones")
        nc.sync.dma_start(ones_sb[:], ones_in[:])
        id_sb = pc.tile([PT, PT], F32, tag="ident")
        nc.sync.dma_start(id_sb[:], ident[:])
        wo_sb = pc.tile([PT, NDC * 256], F32R, tag="wo")
        nc.sync.dma_start(wo_sb[:], wo[:])
        wq_sb = pc.tile([PT, NDC * 256], F32R, tag="wq")
        nc.sync.dma_start(wq_sb[:], wq[:])
        wk_sb = pc.tile([PT, NDC * 256], F32R, tag="wk")
        nc.sync.dma_start(wk_sb[:], wk[:])
        wv_sb = pc.tile([PT, NDC * 256], F32R, tag="wv")
        nc.sync.dma_start(wv_sb[:], wv[:])
        cq_sb = pc.tile([PT, 2], F32, tag="cq")
        nc.sync.dma_start(cq_sb[:], cq[:])
        ck_sb = pc.tile([PT, 2], F32, tag="ck")
        nc.sync.dma_start(ck_sb[:], ck[:])
        cv_sb = pc.tile([1, HPC * E], F32R, tag="cv")
        nc.sync.dma_start(cv_sb[:], cv[:])

        qT = [pc.tile([PT, S], F32R, tag=f"qT{m}", name=f"qT{m}") for m in range(2)]
        kT = [pc.tile([PT, S], F32R, tag=f"kT{m}", name=f"kT{m}") for m in range(2)]
        v_sb = [
            pc.tile([PT, HPC * (E + 1)], F32R, tag=f"v{t}", name=f"v{t}")
            for t in range(NST)
        ]
        for tt in range(NST):
            nc.sync.dma_start(v_sb[tt][:], vinit[:])

        pA = top.enter_context(tc.tile_pool(name="A_sb", bufs=2))
        pSt = top.enter_context(tc.tile_pool(name="A_st", bufs=4))
        pY = top.enter_context(tc.tile_pool(name="Y", bufs=2))
        pCe = top.enter_context(tc.tile_pool(name="C_ex", bufs=3))
        pCt = top.enter_context(tc.tile_pool(name="C_sb", bufs=2))
        pEa = top.enter_context(tc.tile_pool(name="E_at", bufs=9))
        pEo = top.enter_context(tc.tile_pool(name="E_sb", bufs=3))
        # PSUM: big(sc/qk: 3) + tp(2) + aU(1) + med(bc/v/E: 2) = 8 banks
        pPb = top.enter_context(tc.tile_pool(name="P_big", bufs=3, space="PSUM"))
        pPt = top.enter_context(tc.tile_pool(name="P_tp", bufs=2, space="PSUM"))
        pPa = top.enter_context(tc.tile_pool(name="P_aU", bufs=1, space="PSUM"))
        pPm = top.enter_context(tc.tile_pool(name="P_med", bufs=2, space="PSUM"))

        for g in range(NSC):
            # ---------------- A(g): layernorm + transpose ----------------
            yT = [
                pY.tile([PT, SC], F32R, tag=f"yT{c}", name=f"yTg{c}")
                for c in range(NDC)
            ]
            for stl in range(4):
                st = 4 * g + stl
                x_t = pA.tile([PT, D], F32, tag="x")
                nc.sync.dma_start(x_t[:], x[PT * st : PT * (st + 1), :])
                s1 = pSt.tile([PT, 1], F32, tag="s1")
                nc.vector.tensor_reduce(
                    s1[:], x_t[:], axis=mybir.AxisListType.X, op=ALU.add
                )
                sq = pA.tile([PT, D], F32, tag="sq")
                nc.vector.tensor_mul(sq[:], x_t[:], x_t[:])
                ssq = pSt.tile([PT, 1], F32, tag="ssq")
                nc.vector.tensor_reduce(
                    ssq[:], sq[:], axis=mybir.AxisListType.X, op=ALU.add
                )
                nmean = pSt.tile([PT, 1], F32, tag="nm")
                nc.vector.tensor_scalar_mul(nmean[:], s1[:], -1.0 / D)
                ve = pSt.tile([PT, 1], F32, tag="ve")
                nc.vector.tensor_scalar(
                    ve[:], ssq[:], 1.0 / D, EPS, op0=ALU.mult, op1=ALU.add
                )
                m2 = pSt.tile([PT, 1], F32, tag="m2")
                nc.vector.tensor_mul(m2[:], nmean[:], nmean[:])
                va = pSt.tile([PT, 1], F32, tag="va")
                nc.vector.tensor_sub(va[:], ve[:], m2[:])
                std = pSt.tile([PT, 1], F32, tag="std")
                nc.scalar.activation(std[:], va[:], AF.Sqrt)
                istd = pSt.tile([PT, 1], F32, tag="istd")
                nc.vector.reciprocal(istd[:], std[:])
                nmi = pSt.tile([PT, 1], F32, tag="nmi")
                nc.vector.tensor_mul(nmi[:], nmean[:], istd[:])
                y_t = pA.tile([PT, D], F32, tag="y")
                nc.vector.tensor_scalar(
                    y_t[:], x_t[:], istd[:], nmi[:], op0=ALU.mult, op1=ALU.add
                )
                for dc in range(NDC):
                    tp = pPt.tile([PT, PT], F32, tag="tp")
                    nc.tensor.transpose(
                        tp[:], y_t[:, PT * dc : PT * (dc + 1)], id_sb[:]
                    )
                    nc.vector.tensor_copy(
                        yT[dc][:, PT * stl : PT * (stl + 1)], tp[:]
                    )

            # ---------------- B(g): q/k transposed, v natural ----------------
            for w_s, c_s, dst in ((wq_sb, cq_sb, qT), (wk_sb, ck_sb, kT)):
                for m in range(2):
                    ps = pPb.tile([PT, SC], F32, tag="big")
                    for dc in range(NDC):
                        nc.tensor.matmul(
                            ps[:],
                            w_s[:, 256 * dc + PT * m : 256 * dc + PT * (m + 1)],
                            yT[dc][:],
                            start=(dc == 0),
                            stop=(dc == NDC - 1),
                        )
                    nc.vector.tensor_scalar_add(
                        dst[m][:, SC * g : SC * (g + 1)], ps[:], c_s[:, m : m + 1]
                    )
            for stl in range(4):
                tt = 4 * g + stl
                ps = pPm.tile([PT, HPC * E], F32, tag="med")
                for dc in range(NDC):
                    nc.tensor.matmul(
                        ps[:],
                        yT[dc][:, PT * stl : PT * (stl + 1)],
                        wv_sb[:, 256 * dc : 256 * (dc + 1)],
                        start=(dc == 0),
                        stop=False,
                    )
                nc.tensor.matmul(
                    ps[:], ones_sb[0:1, 0:PT], cv_sb[0:1, :],
                    start=False, stop=True,
                )
                vt = v_sb[tt].rearrange("p (h e) -> p h e", e=E + 1)
                nc.vector.tensor_copy(
                    vt[:, :, 0:E], ps.rearrange("p (h e) -> p h e", e=E)[:]
                )

            # ---------- C(j=g): attention + AllGather + output ----------
            j = g
            for h in range(HPC):
                m, o = h // 2, E * (h % 2)
                aU = pPa.tile([E + 1, SC], F32, tag="aU")
                nt = 4 * j + 4
                for i in range(nt):
                    sc = pPb.tile([PT, SC], F32, tag="big")
                    nc.tensor.matmul(
                        sc[:],
                        kT[m][o : o + E, PT * i : PT * (i + 1)],
                        qT[m][o : o + E, SC * j : SC * (j + 1)],
                    )
                    ex = pCe.tile([PT, SC], F32R, tag="ex")
                    nc.scalar.activation(ex[:], sc[:], AF.Exp, scale=0.125)
                    if i >= 4 * j:
                        exm = pCe.tile([PT, SC], F32R, tag="exm")
                        nc.gpsimd.affine_select(
                            exm[:], ex[:], pattern=[[1, SC]],
                            compare_op=ALU.is_ge, fill=0.0,
                            base=SC * j - PT * i, channel_multiplier=-1,
                        )
                        ex = exm
                    nc.tensor.matmul(
                        aU[:],
                        v_sb[i][:, (E + 1) * h : (E + 1) * (h + 1)],
                        ex[:],
                        start=(i == 0),
                        stop=(i == nt - 1),
                    )
                aU_sb = pCt.tile([E + 1, SC], F32, tag="aUs")
                nc.vector.tensor_copy(aU_sb[:], aU[:])
                rc32 = pCt.tile([1, SC], F32, tag="rc32")
                nc.vector.reciprocal(rc32[:], aU_sb[E : E + 1, :])
                rc = pCt.tile([1, SC], F32R, tag="rc")
                nc.vector.tensor_copy(rc[:], rc32[:])
                bc = pPm.tile([E, SC], F32, tag="med")
                nc.tensor.matmul(bc[:], ones_sb[0:1, 0:E], rc[0:1, :])
                aT = pCt.tile([E, SC], F32R, tag="aT")
                nc.vector.tensor_mul(aT[:], aU_sb[0:E, :], bc[:])
                nc.sync.dma_start(cc_in[j][E * h : E * (h + 1), :], aT[:])

            if collective:
                nc.gpsimd.collective_compute(
                    "AllGather",
                    ALU.bypass,
                    replica_groups=GROUPS,
                    ins=[cc_in[j][:]],
                    outs=[cc_out[j][:]],
                )
            else:
                nc.sync.dma_start(cc_out[j][0 : HPC * E, :], cc_in[j][:])

            at = []
            for fc in range(NDC):
                t = pEa.tile([PT, SC], F32R, tag="at", name="at")
                nc.sync.dma_start(t[:], cc_out[j][PT * fc : PT * (fc + 1), :])
                at.append(t)
            for stl in range(4):
                st = 4 * j + stl
                ops = pPm.tile([PT, COLS], F32, tag="med")
                for fc in range(NDC):
                    nc.tensor.matmul(
                        ops[:],
                        at[fc][:, PT * stl : PT * (stl + 1)],
                        wo_sb[:, 256 * fc : 256 * (fc + 1)],
                        start=(fc == 0),
                        stop=(fc == NDC - 1),
                    )
                xr = pEo.tile([PT, COLS], F32, tag="xr")
                nc.sync.dma_start(xr[:], xres[PT * st : PT * (st + 1), :])
                ot = pEo.tile([PT, COLS], F32, tag="ot")
                nc.vector.tensor_add(ot[:], ops[:], xr[:])
                nc.sync.dma_start(out[PT * st : PT * (st + 1), :], ot[:])

    nc.compile()
    return nc


_PROGRAM_CACHE = {}


def _get_program():
    if "nc" not in _PROGRAM_CACHE:
        _PROGRAM_CACHE["nc"] = build_program()
    return _PROGRAM_CACHE["nc"]


def make_in_maps(x, ln_w, ln_b, wq, wk, wv, wo):
    """Host-side sharding: fold LN affine into weights, slice per core."""
    lw = ln_w.astype(np.float64)
    lb = ln_b.astype(np.float64)
    wq64, wk64, wv64 = (w.astype(np.float64) for w in (wq, wk, wv))
    wqf = (wq64 * lw[None, :, None]).astype(np.float32)
    wkf = (wk64 * lw[None, :, None]).astype(np.float32)
    wvf = (wv64 * lw[None, :, None]).astype(np.float32)
    cqf = np.einsum("d,hde->he", lb, wq64).astype(np.float32)
    ckf = np.einsum("d,hde->he", lb, wk64).astype(np.float32)
    cvf = np.einsum("d,hde->he", lb, wv64).astype(np.float32)
    ident = np.eye(PT, dtype=np.float32)
    vinit = np.ones((PT, HPC * (E + 1)), np.float32)

    def chunk(m):  # [1024, 256] -> [128, 8*256]: d-chunk c at cols 256c
        return np.ascontiguousarray(
            m.reshape(NDC, PT, 256).transpose(1, 0, 2).reshape(PT, NDC * 256))

    in_maps = []
    for c in range(8):
        b, r = c // 4, c % 4
        hs = slice(HPC * r, HPC * (r + 1))
        wq_c = chunk(wqf[hs].transpose(1, 0, 2).reshape(D, HPC * E))
        wk_c = chunk(wkf[hs].transpose(1, 0, 2).reshape(D, HPC * E))
        wv_c = chunk(wvf[hs].transpose(1, 0, 2).reshape(D, HPC * E))
        wo_c = chunk(wo[:, COLS * r : COLS * (r + 1)])
        cq_c = np.ascontiguousarray(cqf[hs].reshape(2, PT).T)
        ck_c = np.ascontiguousarray(ckf[hs].reshape(2, PT).T)
        cv_c = cvf[hs].reshape(1, HPC * E)
        in_maps.append(dict(
            x=np.ascontiguousarray(x[b]),
            wq=wq_c, wk=wk_c, wv=wv_c, wo=wo_c,
            cq=cq_c, ck=ck_c, cv=cv_c,
            xres=np.ascontiguousarray(x[b][:, COLS * r : COLS * (r + 1)]),
            ident=ident,
            ones_in=np.ones((1, PT), np.float32),
            vinit=vinit,
        ))
    return in_maps


def assemble(results):
    out = np.empty((B, S, D), dtype=np.float32)
    for c in range(8):
        b, r = c // 4, c % 4
        out[b, :, COLS * r : COLS * (r + 1)] = results[c]["out"]
    return out


def kernel(x, ln_w, ln_b, wq, wk, wv, wo, _trace=False):
    nc = _get_program()
    in_maps = make_in_maps(x, ln_w, ln_b, wq, wk, wv, wo)
    try:
        res = run_bass_kernel_spmd(
            nc, in_maps, core_ids=list(range(8)), trace=_trace
        )
    except ModuleNotFoundError:
        res = run_bass_kernel_spmd(nc, in_maps, core_ids=list(range(8)))
    out = assemble(res.results)
    if _trace:
        kernel.last_result = res
    return out


if __name__ == "__main__":
    rng = np.random.default_rng(0)
    x = rng.standard_normal((B, S, D), dtype=np.float32)
    ln_w = np.ones(D, np.float32)
    ln_b = np.zeros(D, np.float32)
    wq = (rng.random((H, D, E), dtype=np.float32) * 0.02)
    wk = (rng.random((H, D, E), dtype=np.float32) * 0.02)
    wv = (rng.random((H, D, E), dtype=np.float32) * 0.02)
    wo = (rng.random((D, D), dtype=np.float32) * 0.02)
    o = kernel(x, ln_w, ln_b, wq, wk, wv, wo)
    print(o.shape, o.dtype)
